# revision 34
# baseline (speedup 1.0000x reference)
"""Trainium2 Bass kernel for the bilinear block classifier.

logits[n, c] = sum_{k,i,j} W[c, k*4096+i*64+j] * head[n, 64k+i] * tail[n, 64k+j] + b[c]
head/tail [4096, 768] fp32, W [97, 49152] fp32, b [97] fp32.

Data-parallel over 8 NeuronCores (512 samples each). Per core, the 49152-dim
outer-product feature tensor is materialized chunk-by-chunk (384 chunks of
128 features x 512 samples) in [feature, sample] layout, then contracted
against host-reordered W^T chunks with fp32 PSUM accumulation into a single
[97, 512] bank.

Chunks are processed in groups to amortize per-instruction overhead:
  route A (PE-replication), groups of 2: K=64 selection matmuls broadcast two
    head^T rows across 128 partitions each -> one 2-bank fp32 PSUM tile; one
    ScalarE copy evacuates the pair to fp16 SBUF.
  route C (host-replication), groups of 4: the replicated head^T rows arrive
    pre-built from the host in one 512KB DMA.
One fp16 VectorE tensor-multiply per group (2x mode) against a host-built
4x-duplicated tail^T tile forms the outer products. Routes are interleaved
to balance PE / ScalarE / VectorE / DMA. Bias is added during the final
PSUM evacuation; the host reassembles [4096, 97] from per-core [97, 512].
"""

import numpy as np

EMB = 768
BLK = 64
NCLS = 97
NTOT = 4096
NB = 12             # feature blocks of 64
NCORES = 8
NPC = NTOT // NCORES    # 512 samples per core
NM = BLK // 2           # 32 chunks per block (2 i-rows x 64 j each)
NCHUNK = NB * NM        # 384 chunks of 128 features

GA = 2               # chunks per route-A group (PSUM banks per tile)
GC = 4               # chunks per route-C group
# per-k patterns of groups; alternate to average 4.5 C-groups per block
# "C" = 4-chunk host-replicated group, "c" = 2-chunk host-replicated group,
# "A" = 2-chunk PE-replicated group; 18 C-chunks + 14 A-chunks per block
K_PATTERNS = [
    ["C", "A", "C", "A", "C", "A", "C", "A", "C", "A", "A"],
]
N_WARMUP = 0        # PE warmup matmuls issued during the DMA head

_CACHE = {}


def _groups():
    """Yield (route, k, m0, size) for every group, in chunk order."""
    out = []
    for k in range(NB):
        m0 = 0
        for r in K_PATTERNS[k % len(K_PATTERNS)]:
            size = GC if r == "C" else GA
            out.append((r, k, m0, size))
            m0 += size
        assert m0 == NM
    return out


def _split_excess_waits(nc, limit=1):
    """walrus in this toolchain rejects instructions carrying more than
    `limit` semaphore waits; split extras into preceding wait-only Drains."""
    import concourse.mybir as mybir

    n_new = 0
    for bb in nc.main_func.blocks:
        new_list = []
        for ins in bb.instructions:
            si = ins.sync_info
            if si is not None and si.on_wait and len(si.on_wait) > limit:
                waits = list(si.on_wait)
                extra, keep = waits[:-limit], waits[-limit:]
                for i in range(0, len(extra), limit):
                    chunk = extra[i : i + limit]
                    n_new += 1
                    d = mybir.InstDrain(
                        name=f"I-waitsplit-{n_new}",
                        engine=ins.engine,
                        ins=[],
                        outs=[],
                        sync_info=mybir.SyncInfo(on_wait=chunk, on_update=[]),
                    )
                    nc.register_instruction(d)
                    new_list.append(d)
                si.on_wait = keep
            new_list.append(ins)
        bb.instructions[:] = new_list
    return n_new


def _build_nc():
    import concourse.bass as bass
    import concourse.mybir as mybir
    import concourse.tile as tile

    dt = mybir.dt
    nc = bass.Bass()

    groups = _groups()
    n_c4 = sum(1 for g in groups if g[0] == "C")
    n_c2 = sum(1 for g in groups if g[0] == "c")

    napk = sum(GA for r in K_PATTERNS[0] if r == "A")   # A-pairs per block
    nks = (NB + 3) // 4                                  # k-slots per base
    b1p = nc.dram_tensor(
        "b1p", [8, nks * napk * NPC], dt.float16, kind="ExternalInput"
    )
    b2d = nc.dram_tensor(
        "b2d", [128, NB * 2 * NPC], dt.float16, kind="ExternalInput"
    )
    wt = nc.dram_tensor("wt", [NB, 128, NM * NCLS], dt.float16, kind="ExternalInput")
    bia = nc.dram_tensor("bias", [NCLS, 1], dt.float32, kind="ExternalInput")
    s2 = nc.dram_tensor("s2", [128, 128], dt.float16, kind="ExternalInput")
    b1r = nc.dram_tensor(
        "b1r", [n_c4, 128, GC * NPC], dt.float16, kind="ExternalInput"
    )
    b1r2 = nc.dram_tensor(
        "b1r2", [max(n_c2, 1), 128, GA * NPC], dt.float16, kind="ExternalInput"
    )
    out = nc.dram_tensor("logits_t", [NCLS, NPC], dt.float32, kind="ExternalOutput")

    with tile.TileContext(nc) as tc:
        with (
            tc.tile_pool(name="cst", bufs=1) as cst,
            tc.tile_pool(name="wp", bufs=4) as wp,
            tc.tile_pool(name="r1a", bufs=5) as r1a,
            tc.tile_pool(name="r1c", bufs=7) as r1c,
            tc.tile_pool(name="blta", bufs=5) as blta,
            tc.tile_pool(name="bltc", bufs=7) as bltc,
            tc.tile_pool(name="ps", bufs=6, space="PSUM") as ps,
            tc.tile_pool(name="accp", bufs=1, space="PSUM") as accp,
            tc.tile_pool(name="wup", bufs=1, space="PSUM") as wup,
        ):
            b1sb = cst.tile([128, nks * napk * NPC], dt.float16, tag="b1")
            b2sb = cst.tile([128, NB * 2 * NPC], dt.float16, tag="b2")
            ssb = cst.tile([128, 128], dt.float16, tag="s2")
            biasb = cst.tile([NCLS, 1], dt.float32, tag="bias")
            lgsb = cst.tile([NCLS, NPC], dt.float32, tag="logits")

            nc.sync.dma_start(ssb[:, :], s2[:, :])
            for bi in range(4):
                nc.sync.dma_start(
                    b1sb[32 * bi : 32 * bi + 2, :], b1p[2 * bi : 2 * bi + 2, :]
                )
            nc.sync.dma_start(biasb[:, :], bia[:, :])

            if N_WARMUP:
                # keep the PE p-state warm while input DMAs land
                wups = wup.tile([64, NPC], dt.float32)
                for _ in range(N_WARMUP):
                    nc.tensor.matmul(
                        wups[:, :],
                        ssb[0:64, 0:64],
                        ssb[0:64, 0:NPC],
                        start=True,
                        stop=True,
                        skip_group_check=True,
                    )

            # two independent accumulation chains so the DMA-fed (C) and
            # PE-fed (A) pipelines never serialize each other
    
            acc_a = accp.tile([NCLS, NPC], dt.float32, tag="acc_a")
            acc_c = accp.tile([NCLS, NPC], dt.float32, tag="acc_c")
            routes = {}
            for (route, k, m0, size) in groups:
                for g in range(size):
                    routes[k * NM + m0 + g] = "A" if route == "A" else "C"
            a_chunks = [c for c in sorted(routes) if routes[c] == "A"]
            c_chunks = [c for c in sorted(routes) if routes[c] == "C"]
            bounds = {
                "A": (a_chunks[0], a_chunks[-1]),
                "C": (c_chunks[0], c_chunks[-1]),
            }
            ci = 0
            ci2 = 0
            chunk = 0
            apair = {k: 0 for k in range(NB)}
            wtiles = {}

            def stage1(route, k, m0, size):
                nonlocal ci, ci2
                if k not in wtiles:
                    # issue this k-block's W and tail tiles just-in-time so
                    # early route-C DMAs aren't queued behind all of W
                    nc.sync.dma_start(
                        b2sb[:, k * 2 * NPC : (k + 1) * 2 * NPC],
                        b2d[:, k * 2 * NPC : (k + 1) * 2 * NPC],
                    )
                    wk = wp.tile([128, NM * NCLS], dt.float16, tag="wk")
                    nc.sync.dma_start(wk[:, :], wt[k])
                    wtiles[k] = wk
                if route == "C":
                    r1sb = r1c.tile([128, GC * NPC], dt.float16, tag="r1c")
                    nc.sync.dma_start(r1sb[:, :], b1r[ci])
                    ci += 1
                elif route == "c":
                    r1sb = r1c.tile([128, GA * NPC], dt.float16, tag="r1c")
                    nc.sync.dma_start(r1sb[:, :], b1r2[ci2])
                    ci2 += 1
                else:
                    r1sb = r1a.tile([128, GA * NPC], dt.float16, tag="r1a")
                    b = (k % 4) * 32
                    for g in range(GA):
                        off = ((k // 4) * napk + apair[k]) * NPC
                        apair[k] += 1
                        r1ps = ps.tile([128, NPC], dt.float32, tag="r1ps")
                        nc.tensor.matmul(
                            r1ps[:, :],
                            ssb[b : b + 2, 0:128],
                            b1sb[b : b + 2, off : off + NPC],
                            start=True,
                            stop=True,
                            skip_group_check=True,
                            tile_position=(b, 0),
                        )
                        nc.scalar.copy(
                            r1sb[:, g * NPC : (g + 1) * NPC], r1ps[:, :]
                        )
                return r1sb

            def stage2(route, k, m0, size, r1sb, chunk0):
                pool = bltc if route in ("C", "c") else blta
                tag = "bltc" if route in ("C", "c") else "blta"
                blt = pool.tile([128, size * NPC], dt.float16, tag=tag)
                b2slice = b2sb[:, k * 2 * NPC : (k + 1) * 2 * NPC]
                for h in range(0, size, 2):
                    nc.vector.tensor_mul(
                        blt[:, h * NPC : (h + 2) * NPC],
                        r1sb[:, h * NPC : (h + 2) * NPC],
                        b2slice,
                    )
                wsb = wtiles[k]
                acc = acc_a if route == "A" else acc_c
                first, last = bounds["A" if route == "A" else "C"]
                for g in range(size):
                    c = chunk0 + g
                    cl = m0 + g
                    nc.tensor.matmul(
                        acc[:, :],
                        wsb[:, cl * NCLS : (cl + 1) * NCLS],
                        blt[:, g * NPC : (g + 1) * NPC],
                        start=(c == first),
                        stop=(c == last),
                        skip_group_check=True,
                    )

            SKEW = 1
            pending = []
            for (route, k, m0, size) in groups:
                r1sb = stage1(route, k, m0, size)
                pending.append((route, k, m0, size, r1sb, chunk))
                chunk += size
                if len(pending) > SKEW:
                    stage2(*pending.pop(0))
            for p in pending:
                stage2(*p)
            import concourse.mybir as _mybir
            acc_a_sb = cst.tile([NCLS, NPC], dt.float32, tag="acc_a_sb")
            nc.scalar.copy(acc_a_sb[:, :], acc_a[:, :])
            nc.vector.scalar_tensor_tensor(
                lgsb[:, :],
                acc_c[:, :],
                biasb[:, :],
                acc_a_sb[:, :],
                op0=_mybir.AluOpType.add,
                op1=_mybir.AluOpType.add,
            )
            nc.sync.dma_start(out[:, :], lgsb[:, :])

    _split_excess_waits(nc, limit=1)
    return nc


def _prep_shared(W, b):
    # W [97, 49152] -> wt [12, 128, 32*97] fp16; chunk (k, m) partition
    # p = di*64 + j corresponds to W[c, k, 2m+di, j].
    Wr = np.asarray(W, np.float32).reshape(NCLS, NB, NM, 2, BLK)
    wt = (
        Wr.transpose(3, 4, 1, 2, 0)  # [di, j, k, m, c]
        .reshape(128, NB, NM * NCLS)
        .transpose(1, 0, 2)
        .astype(np.float16)
    )
    bias = np.asarray(b, np.float32).reshape(NCLS, 1)
    # s2[b+q, p] = 1 iff q == p//64, for each base b in {0,32,64,96}
    s2 = np.zeros((128, 128), np.float16)
    for base in (0, 32, 64, 96):
        s2[base, :64] = 1.0
        s2[base + 1, 64:] = 1.0
    return np.ascontiguousarray(wt), bias, s2


def _prep_core(head, tail, groups):
    b1T = np.asarray(head, np.float32).T.astype(np.float16)  # [768, NPC]
    napk = sum(2 for r in K_PATTERNS[0] if r == "A")
    nks = (NB + 3) // 4
    b1p = np.zeros((8, nks * napk * NPC), np.float16)
    apair = {k: 0 for k in range(NB)}
    for (route, k, m0, size) in groups:
        if route != "A":
            continue
        bi = k % 4
        for g in range(size):
            m = m0 + g
            s = (k // 4) * napk + apair[k]
            apair[k] += 1
            for d in (0, 1):
                b1p[2 * bi + d, s * NPC : (s + 1) * NPC] = b1T[64 * k + 2 * m + d]
    b2T = np.asarray(tail, np.float32).T.astype(np.float16).reshape(NB, BLK, NPC)
    b2dup = np.concatenate([b2T, b2T], axis=1)  # [12, 128, NPC]
    b2d = (
        np.broadcast_to(b2dup[:, None], (NB, 2, 128, NPC))
        .transpose(2, 0, 1, 3)
        .reshape(128, NB * 2 * NPC)
    )
    # host-replicated groups: b1r[gi, p, g*NPC + n] = b1T[64k + 2(m0+g) + p//64, n]
    pairs = b1T.reshape(NB, NM, 2, NPC)
    b1r_list, b1r2_list = [], []
    for (route, k, m0, size) in groups:
        if route == "A":
            continue
        sel = pairs[k, m0 : m0 + size]          # [size, 2, NPC]
        rep = np.repeat(sel, 64, axis=1)        # [size, 128, NPC]
        arr = rep.transpose(1, 0, 2).reshape(128, size * NPC)
        (b1r_list if route == "C" else b1r2_list).append(arr)
    b1r = np.stack(b1r_list, axis=0)
    if b1r2_list:
        b1r2 = np.stack(b1r2_list, axis=0)
    else:
        b1r2 = np.zeros((1, 128, GA * NPC), np.float16)
    return (
        b1p,
        np.ascontiguousarray(b2d),
        np.ascontiguousarray(b1r),
        np.ascontiguousarray(b1r2),
    )


def kernel(head_embeddings, tail_embeddings, W, b):
    from concourse.bass_utils import run_bass_kernel_spmd

    assert head_embeddings.shape == (NTOT, EMB), head_embeddings.shape
    assert tail_embeddings.shape == (NTOT, EMB), tail_embeddings.shape
    assert W.shape == (NCLS, EMB * BLK), W.shape

    if "nc" not in _CACHE:
        _CACHE["nc"] = _build_nc()
    nc = _CACHE["nc"]

    groups = _groups()
    wt, bias, s2 = _prep_shared(W, b)
    in_maps = []
    for i in range(NCORES):
        s = slice(i * NPC, (i + 1) * NPC)
        b1p, b2d, b1r, b1r2 = _prep_core(
            head_embeddings[s], tail_embeddings[s], groups
        )
        in_maps.append(
            {
                "b1p": b1p,
                "b2d": b2d,
                "wt": wt,
                "bias": bias,
                "s2": s2,
                "b1r": b1r,
                "b1r2": b1r2,
            }
        )

    res = run_bass_kernel_spmd(nc, in_maps, list(range(NCORES)))
    _CACHE["last_results"] = res
    logits = np.concatenate(
        [res.results[i]["logits_t"].T for i in range(NCORES)], axis=0
    )
    return logits.astype(np.float32)
